# revision 33
# baseline (speedup 1.0000x reference)
"""Trainium2 Bass kernel for nn_BoxEncoder (B=128, T=200, NC=3, NB=2, D=512, DH=256).

Strategy (data-parallel over batch, 16 batch items per core x 8 cores):

 - Token layout per core: partition p = bt*8 + q; j-slots 0..149 are box
   tokens (output rows 600 + q*150 + j), slots 150..224 are dist tokens
   (output rows q*75 + (j-150)).
 - All per-box scalars live as 32 feature columns per j-slot in a bf16
   T_feat tile [128, 225*32]; PE transposes of [128,128] chunks give
   feature-major lhsT blocks (cta).
 - LayerNorm stats WITHOUT materializing z: with L = chol(W1p@W1p.T/256)
   and m = W1p.sum(1)/256, one K=128 matmul per chunk against a
   block-diagonal [L|m] rhs yields u (10 cols) + mu per token;
   var = sum(u^2) - mu^2. DVE square+reduce finishes the stats.
 - rstd is folded into scaled feature columns f18..f27 (= geom * rstd)
   plus f28 = -mu*rstd, re-transposed (T2, box chunks only) into cta2.
 - h^T is computed weight-stationary: z_n^T = W1n^T @ x_n^T with N=512
   token columns per matmul, exact GELU applied straight out of PSUM into
   a persistent bf16 hT buffer [256(dh) x 19456(tok)] - no per-token
   transposes and no second z pass.
 - Per box token tile: out = hT0^T@W2hi + hT1^T@W2lo + raw_feats@W2x[cam]
   (K=32 extras fold cat one-hots, conf, center, b2+cam (presence-gated),
   missing_emb ((1-presence)-gated)). Missing boxes produce exactly
   missing_emb (their geometry path is gelu(0)=0).
 - dist tokens are extras-only matmuls (N=512, K=32).
 - Output staged to SBUF as bf16 (host upcasts to f32) halving HBM
   traffic; staging copies rotate across DVE/ACT/GPSIMD.
 - Phases are ordered to keep the tensor engine continuously busy so it
   ramps to its max p-state: T1 -> stats -> dist matmuls (while DVE does
   the stats postprocessing) -> T2 -> interleaved [main(b-1) | z_n^T(b)]
   blocks of 16 slots.
"""

import numpy as np
import ml_dtypes

B, T, NCAM, NB, D, DH = 128, 200, 3, 2, 512, 256
IW, IH = 640.0, 400.0
NCORES = 8
BPC = B // NCORES            # batch items per core
JB, JD = 150, 75             # box / dist j-slots per partition
J = JB + JD                  # 225
F = 32                       # feature columns per j-slot
NCH = (J * F + 127) // 128   # 57 transpose chunks (56 full + 1 of 32 cols)
NCH2 = 38                    # chunks re-transposed for the scaled features
NSLOT = NCH2 * 4             # 152 slots covered by stats / hT (150 box + 2)

_CACHE = {}


def _build_nc():
    from contextlib import ExitStack
    import concourse.bacc as bacc
    import concourse.mybir as mybir
    import concourse.tile as tile

    f32 = mybir.dt.float32
    bf16 = mybir.dt.bfloat16
    A = mybir.AluOpType
    AF = mybir.ActivationFunctionType
    AX = mybir.AxisListType

    # bf16 pack column offsets
    C_ID = 0
    C_W1N = 128                   # [128, 256] W1n tiled 4x (dh0 | dh1)
    C_W2HI = C_W1N + 256
    C_W2LO = C_W2HI + 512
    C_W2X = C_W2LO + 512          # 3 cam variants, 512 each
    C_LM = C_W2X + 3 * 512        # block-diag [L|m], 44 cols
    NBF = C_LM + 44

    nc = bacc.Bacc("TRN2", target_bir_lowering=False, debug=False,
                   num_devices=NCORES)
    fpk = nc.declare_dram_parameter("fpk", [128, 900], f32, isOutput=False)
    bpk = nc.declare_dram_parameter("bpk", [128, NBF], bf16, isOutput=False)
    outd = nc.declare_dram_parameter("outd", [BPC, 600, D], bf16, isOutput=True)
    outb = nc.declare_dram_parameter("outb", [BPC, 1200, D], bf16, isOutput=True)

    with ExitStack() as ctx:
        tc = ctx.enter_context(tile.TileContext(nc))
        cp = ctx.enter_context(tc.tile_pool(name="const", bufs=1))
        sc = ctx.enter_context(tc.tile_pool(name="scratch", bufs=1))
        # PSUM is bank-granular (8 banks). znp(3) spans all phases; each
        # phase scope adds its own pool (ctp 2 / ope 5 / op 5 <= 5 banks).
        znp = ctx.enter_context(tc.tile_pool(name="znps", bufs=3, space="PSUM"))
        bstg = ctx.enter_context(tc.tile_pool(name="bstage", bufs=3))
        dstg = ctx.enter_context(tc.tile_pool(name="dstage", bufs=3))

        fpack = cp.tile([128, 900], f32)
        nc.sync.dma_start(fpack[:], fpk[:])
        bpack = cp.tile([128, NBF], bf16)
        nc.sync.dma_start(bpack[:], bpk[:])

        raw = fpack[:, 0:900]
        idb = bpack[:, C_ID:C_ID + 128]
        w1n = bpack[:, C_W1N:C_W1N + 256]
        w2hi = bpack[:, C_W2HI:C_W2HI + 512]
        w2lo = bpack[:, C_W2LO:C_W2LO + 512]
        w2x = [bpack[:, C_W2X + c * 512: C_W2X + (c + 1) * 512] for c in range(3)]
        lm = bpack[:, C_LM:C_LM + 44]

        TF = cp.tile([128, J * F], bf16)
        nc.gpsimd.memset(TF[:], 0.0)

        TFj = TF.rearrange("p (j f) -> p j f", f=F)
        TFb = TFj[:, :JB, :]                       # box slots
        TFd = TFj[:, JB:, :]                       # dist slots
        TFbp = TF[:, :JB * F].rearrange("p (m g f) -> p m g f", g=2, f=F)
        raw6 = raw.rearrange("p (b s) -> p b s", s=6)
        rawp = raw.rearrange("p (m g s) -> p m g s", g=2, s=6)

        # ---------------- P1: feature planes (DVE, f32 scratch) ----------------
        sPres = sc.tile([128, JB], f32)
        sKey = sc.tile([128, JB], f32)
        sSwap = sc.tile([128, JD], f32)
        sD = sc.tile([128, JD], f32)
        sSD = sc.tile([128, JD], f32)
        sw = [sc.tile([128, JB], f32, tag=f"swp{i}", name=f"swp{i}")
              for i in range(6)]
        sT0 = sc.tile([128, JB], f32)
        sT1 = sc.tile([128, JB], f32)

        nc.vector.tensor_tensor(sT0[:], raw6[:, :, 0], raw6[:, :, 1], A.add)
        nc.vector.tensor_tensor(sT1[:], raw6[:, :, 2], raw6[:, :, 3], A.add)
        nc.vector.tensor_tensor(sT0[:], sT0[:], sT1[:], A.add)
        nc.vector.tensor_scalar(sPres[:], sT0[:], 0.0, None, A.not_equal)
        # key = cat - 1000*pres  (order-equivalent to cat + 1000*(1-pres))
        nc.vector.scalar_tensor_tensor(sKey[:], sPres[:], -1000.0,
                                       raw6[:, :, 4], A.mult, A.add)
        sKeyp = sKey.rearrange("p (m g) -> p m g", g=2)
        nc.vector.tensor_tensor(sSwap[:], sKeyp[:, :, 1], sKeyp[:, :, 0], A.is_lt)

        # compare-and-swap each of the 6 raw components + presence
        # (even components on DVE, odd on GpSimd, with separate scratch)
        sDg = sc.tile([128, JD], f32)
        sSDg = sc.tile([128, JD], f32)
        for i in range(6):
            ve, vo = rawp[:, :, 0, i], rawp[:, :, 1, i]
            dst = sw[i].rearrange("p (m g) -> p m g", g=2)
            if i % 2 == 0:
                e, eD, eSD = nc.vector, sD, sSD
            else:
                e, eD, eSD = nc.gpsimd, sDg, sSDg
            e.tensor_tensor(eD[:], vo, ve, A.subtract)
            e.tensor_tensor(eSD[:], eD[:], sSwap[:], A.mult)
            e.tensor_tensor(dst[:, :, 0], ve, eSD[:], A.add)
            e.tensor_tensor(dst[:, :, 1], vo, eSD[:], A.subtract)
        sPresP = sPres.rearrange("p (m g) -> p m g", g=2)
        nc.vector.tensor_tensor(sD[:], sPresP[:, :, 1], sPresP[:, :, 0], A.subtract)
        nc.vector.tensor_tensor(sSD[:], sD[:], sSwap[:], A.mult)
        nc.vector.tensor_tensor(TFbp[:, :, 0, 14], sPresP[:, :, 0], sSD[:], A.add)
        nc.vector.tensor_tensor(TFbp[:, :, 1, 14], sPresP[:, :, 1], sSD[:], A.subtract)

        sX1, sY1, sX2, sY2, sCat, sConf = sw
        # f0..f3: normalized coords
        nc.vector.tensor_scalar(TFb[:, :, 0], sX1[:], 1.0 / IW, None, A.mult)
        nc.vector.tensor_scalar(TFb[:, :, 1], sY1[:], 1.0 / IH, None, A.mult)
        nc.vector.tensor_scalar(TFb[:, :, 2], sX2[:], 1.0 / IW, None, A.mult)
        nc.vector.tensor_scalar(TFb[:, :, 3], sY2[:], 1.0 / IH, None, A.mult)
        # f4 w, f5 h, f6 cx*2, f7 cy*2 (the 0.5 is folded into the weights)
        # w/h/area/aspect computed in f32 scratch: the aspect denominator
        # h+1e-6 would flip sign under bf16 rounding of h near -1e-6.
        sWn = sc.tile([128, JB], f32)
        sHn = sc.tile([128, JB], f32)
        nc.vector.tensor_tensor(sWn[:], sX2[:], sX1[:], A.subtract)
        nc.vector.tensor_scalar(sWn[:], sWn[:], 1.0 / IW, None, A.mult)
        nc.vector.tensor_tensor(sHn[:], sY2[:], sY1[:], A.subtract)
        nc.vector.tensor_scalar(sHn[:], sHn[:], 1.0 / IH, None, A.mult)
        nc.gpsimd.tensor_copy(TFb[:, :, 4], sWn[:])
        nc.gpsimd.tensor_copy(TFb[:, :, 5], sHn[:])
        nc.gpsimd.tensor_tensor(TFb[:, :, 6], TFb[:, :, 0], TFb[:, :, 2], A.add)
        nc.gpsimd.tensor_tensor(TFb[:, :, 7], TFb[:, :, 1], TFb[:, :, 3], A.add)
        # f8 area, f9 aspect = w / (h + 1e-6)
        nc.vector.tensor_tensor(TFb[:, :, 8], sWn[:], sHn[:], A.mult)
        sHp = sT0
        nc.vector.tensor_scalar(sHp[:], sHn[:], 1e-6, None, A.add)
        sR = sT1
        nc.vector.reciprocal(sR[:], sHp[:])
        nc.vector.tensor_tensor(TFb[:, :, 9], sWn[:], sR[:], A.mult)
        # f10..12 cat one-hots * pres ; f13 conf*pres ; f15 = 1-pres
        for k in range(3):
            nc.vector.scalar_tensor_tensor(TFb[:, :, 10 + k], sCat[:], float(k),
                                           TFb[:, :, 14], A.is_equal, A.mult)
        nc.gpsimd.tensor_tensor(TFb[:, :, 13], sConf[:], TFb[:, :, 14], A.mult)
        nc.vector.tensor_scalar(TFb[:, :, 15], TFb[:, :, 14], -1.0, 1.0,
                                A.mult, A.add)
        # dist tokens: f16 = 0.5*sqrt(dx2^2+dy2^2) (cx stored doubled), f17 = 1
        sDx = sc.tile([128, JD], f32)
        sDy = sc.tile([128, JD], f32)
        nc.vector.tensor_tensor(sDx[:], TFbp[:, :, 0, 6], TFbp[:, :, 1, 6], A.subtract)
        nc.vector.tensor_tensor(sDy[:], TFbp[:, :, 0, 7], TFbp[:, :, 1, 7], A.subtract)
        nc.vector.tensor_tensor(sDx[:], sDx[:], sDx[:], A.mult)
        nc.vector.tensor_tensor(sDy[:], sDy[:], sDy[:], A.mult)
        nc.vector.tensor_tensor(sDx[:], sDx[:], sDy[:], A.add)
        nc.scalar.activation(TFd[:, :, 16], sDx[:], AF.Sqrt, scale=0.25)
        nc.vector.memset(TFd[:, :, 17], 1.0)

        hT = [cp.tile([128, NSLOT * 128], bf16, tag=f"hT{i}", name=f"hT{i}")
              for i in range(2)]
        hT4 = [h.rearrange("p (s4 jj q) -> p s4 jj q", jj=4, q=128) for h in hT]
        cta = cp.tile([128, NCH * 128], bf16)
        cta2 = cp.tile([128, NCH2 * 128], bf16)
        vd = outd.rearrange("b (q r) d -> b q r d", q=8)
        vb = outb.rearrange("b (q r) d -> b q r d", q=8)
        NBLK = (NSLOT + 15) // 16          # 10 blocks; block 9 is half-size

        def zn_mm(b, k):
            # z_n^T matmul k (dhc=k//4, jj=k%4) of slot-block b + exact GELU
            dhc, jj = k // 4, k % 4
            c0 = b * 512
            nb = min(512, NCH2 * 128 - c0)
            tcnt = nb // 128
            zt = znp.tile([128, 512], f32, tag="zn")
            nc.tensor.matmul(
                zt[:, :nb],
                w1n[32 * jj:32 * jj + 32, dhc * 128:(dhc + 1) * 128],
                cta2[32 * jj:32 * jj + 32, c0:c0 + nb],
                start=True, stop=True, tile_position=(32 * jj, 0))
            ztv = zt.rearrange("p (t q) -> p t q", q=128)
            nc.scalar.activation(hT4[dhc][:, 4 * b:4 * b + tcnt, jj, :],
                                 ztv[:, 0:tcnt, :], AF.Gelu)

        with tc.tile_pool(name="ctps", bufs=2, space="PSUM") as ctp:
            # -------- T1: transpose T_feat chunks -> bf16 lhsT tiles --------
            for ci in range(NCH):
                w_cols = min(128, J * F - ci * 128)
                ps = ctp.tile([128, 128], bf16, tag="ct")
                nc.tensor.transpose(ps[:w_cols, :],
                                    TF[:, ci * 128: ci * 128 + w_cols], idb)
                dst = cta[:w_cols, ci * 128: ci * 128 + 128]
                if ci % 3 == 2:
                    nc.scalar.copy(dst, ps[:w_cols, :])
                else:
                    nc.vector.tensor_copy(dst, ps[:w_cols, :])

        with tc.tile_pool(name="opse", bufs=5, space="PSUM") as ope:
            # -------- stats: u/mu per token via [L|m] matmuls ---------------
            stats_sb = sc.tile([128, NCH2 * 44], f32)
            for ci in range(NCH2):
                sm = ope.tile([128, D], f32, tag="o")
                nc.tensor.matmul(sm[:, 0:44], cta[:, ci * 128:(ci + 1) * 128],
                                 lm, start=True, stop=True)
                nc.vector.tensor_copy(stats_sb[:, ci * 44:(ci + 1) * 44],
                                      sm[:, 0:44])

            # DVE post: var = sum(u^2) - mu^2 ; rstd = 1/sqrt(var+eps)
            sq = sc.tile([128, NCH2 * 44], f32)
            nc.vector.tensor_tensor(sq[:], stats_sb[:], stats_sb[:], A.mult)
            sqv = sq.rearrange("p (s e) -> p s e", e=11)
            stv = stats_sb.rearrange("p (s e) -> p s e", e=11)
            usum = sc.tile([128, NSLOT], f32)
            nc.vector.tensor_reduce(usum[:], sqv[:, :, 0:10], AX.X, A.add)
            var = sc.tile([128, NSLOT], f32)
            nc.vector.tensor_tensor(var[:], usum[:], sqv[:, :, 10], A.subtract)
            eps = sc.tile([128, 1], f32)
            nc.vector.memset(eps[:], 1e-5)
            sd = sc.tile([128, NSLOT], f32)
            nc.scalar.activation(sd[:], var[:], AF.Sqrt, bias=eps[:])
            rstd = sc.tile([128, NSLOT], f32)
            nc.vector.reciprocal(rstd[:], sd[:])
            # scaled features f18..f27 = geom * rstd ; f28 = -mu*rstd
            for i in range(10):
                eng_i = nc.vector if i % 2 == 0 else nc.gpsimd
                eng_i.tensor_tensor(TFb[:, :, 18 + i], TFb[:, :, i],
                                    rstd[:, 0:JB], A.mult)
            nc.vector.scalar_tensor_tensor(TFb[:, :, 28], stv[:, 0:JB, 10],
                                           -1.0, rstd[:, 0:JB], A.mult, A.mult)

            # -------- dist tokens (deep ope pool hides the staging casts) ---
            dist_stage = None
            for dk in range(JD):
                j = JB + dk
                ci, jj = j // 4, j % 4
                o = ope.tile([128, D], f32, tag="o")
                nc.tensor.matmul(o[:],
                                 cta[32 * jj:32 * jj + 32,
                                     ci * 128:(ci + 1) * 128],
                                 w2x[0][32 * jj:32 * jj + 32, :],
                                 start=True, stop=True,
                                 tile_position=(32 * jj, 0))
                if dist_stage is None:
                    dist_stage = dstg.tile([128, 8 * D], bf16, tag="dstage")
                slot = dk % 8
                dst = dist_stage[:, slot * D:(slot + 1) * D]
                # first copies go to ACT so DVE can drain the stats
                # postprocessing chain without starving the ope pool
                if dk < 12 or dk % 2 == 0:
                    nc.scalar.copy(dst, o[:])
                else:
                    nc.vector.tensor_copy(dst, o[:])
                if slot == 7 or dk == JD - 1:
                    g = slot + 1
                    nc.sync.dma_start(vd[:, :, dk - g + 1: dk + 1, :],
                                      dist_stage[:, : g * D])
                    dist_stage = None

        with tc.tile_pool(name="ctps2", bufs=2, space="PSUM") as ctp:
            # -------- T2: re-transpose box chunks; weave z_n block 0 in ----
            for ci in range(NCH2):
                ps = ctp.tile([128, 128], bf16, tag="ct")
                nc.tensor.transpose(ps[:], TF[:, ci * 128:(ci + 1) * 128], idb)
                dst = cta2[:, ci * 128:(ci + 1) * 128]
                if ci % 3 == 2:
                    nc.scalar.copy(dst, ps[:])
                else:
                    nc.vector.tensor_copy(dst, ps[:])
                if 4 <= ci < 12:
                    zn_mm(0, ci - 4)

        # -------- main box loop; z_n of block b+1 woven into block b --------
        # Three slots' accumulation groups are interleaved so consecutive PE
        # instructions hit different PSUM banks and pipeline-overlap.
        with tc.tile_pool(name="ops", bufs=5, space="PSUM") as op:
            box_stage = None
            gstart = 0
            eng = 0
            for j0 in range(0, JB, 3):
                grp = (j0, j0 + 1, j0 + 2)
                ot = {j: op.tile([128, D], f32, tag="o", name=f"o{j}")
                      for j in grp}
                for j in grp:
                    nc.tensor.matmul(ot[j][:], hT[0][:, j * 128:(j + 1) * 128],
                                     w2hi, start=True, stop=False)
                for j in grp:
                    nc.tensor.matmul(ot[j][:], hT[1][:, j * 128:(j + 1) * 128],
                                     w2lo, start=False, stop=False)
                for j in grp:
                    ci, jj = j // 4, j % 4
                    cam = (j % 6) // 2
                    nc.tensor.matmul(ot[j][:], cta[32 * jj:32 * jj + 32,
                                               ci * 128:(ci + 1) * 128],
                                     w2x[cam][32 * jj:32 * jj + 32, :],
                                     start=False, stop=True,
                                     tile_position=(32 * jj, 0))
                for j in grp:
                    if j % 2 == 0 and j // 16 + 1 < NBLK:
                        zn_mm(j // 16 + 1, (j % 16) // 2)
                for j in grp:
                    if box_stage is None:
                        box_stage = bstg.tile([128, 8 * D], bf16, tag="bstage")
                        gstart = j
                    off = j - gstart
                    dst = box_stage[:, off * D:(off + 1) * D]
                    if eng == 0:
                        nc.vector.tensor_copy(dst, ot[j][:])
                    else:
                        nc.scalar.copy(dst, ot[j][:])
                    eng = (eng + 1) % 2
                    if off == 7 or j == JB - 1 or (j >= 144 and off >= 1):
                        nc.sync.dma_start(vb[:, :, gstart: j + 1, :],
                                          box_stage[:, : (off + 1) * D])
                        box_stage = None

    nc.compile()
    return nc


def _prep_inputs(inputs):
    f32 = np.float32
    bf = ml_dtypes.bfloat16
    scale = float(np.asarray(inputs["scale"]))

    # W1 with the cx/cy doubling folded in (features store 2*cx, 2*cy)
    W1p = np.asarray(inputs["geom_w1"], f32).copy()
    W1p[6] *= 0.5
    W1p[7] *= 0.5

    # W1n: rows f18..f27 -> W1p, f28 -> ones (the -mu*rstd passthrough)
    W1n = np.zeros((32, DH), f32)
    W1n[18:28] = W1p
    W1n[28] = 1.0
    w1n_cols = np.concatenate([W1n[:, :128], W1n[:, 128:]], axis=1)  # [32, 256]
    w1n_rep = np.tile(w1n_cols, (4, 1))

    W2s = scale * np.asarray(inputs["geom_w2"], f32)
    w2hi, w2lo = W2s[:128], W2s[128:]

    cat_t = np.asarray(inputs["cat_table"], f32)
    cam_t = np.asarray(inputs["cam_table"], f32)
    bias_row = (np.asarray(inputs["geom_b2"], f32)
                + np.asarray(inputs["conf_b"], f32)
                + np.asarray(inputs["center_b"], f32))
    w2x_reps = []
    for c in range(3):
        W2x = np.zeros((32, D), f32)
        W2x[6] = scale * np.asarray(inputs["center_w"], f32)[0] * 0.5
        W2x[7] = scale * np.asarray(inputs["center_w"], f32)[1] * 0.5
        W2x[10:13] = scale * cat_t
        W2x[13] = scale * np.asarray(inputs["conf_w"], f32)[0]
        W2x[14] = scale * (bias_row + cam_t[c])
        W2x[15] = np.asarray(inputs["missing_emb"], f32)[0]
        W2x[16] = np.asarray(inputs["dist_w"], f32)[0]
        W2x[17] = np.asarray(inputs["dist_b"], f32)
        w2x_reps.append(np.tile(W2x, (4, 1)))

    # stats: L = chol(W1p W1p^T / 256), m = W1p.sum(1)/256;
    # block-diag rhs [128, 44]: rows 32j'+f (f<10) -> cols 11j'+(L[f,:]|m[f])
    G = (W1p @ W1p.T) / float(DH)
    Lc = np.linalg.cholesky(G + 1e-12 * np.eye(10))
    m = W1p.sum(axis=1) / float(DH)
    LM = np.zeros((128, 44), f32)
    for jp in range(4):
        LM[32 * jp:32 * jp + 10, 11 * jp:11 * jp + 10] = Lc
        LM[32 * jp:32 * jp + 10, 11 * jp + 10] = m

    idf32 = np.eye(128, dtype=f32)
    bpk = np.concatenate(
        [idf32, w1n_rep, w2hi, w2lo] + w2x_reps + [LM], axis=1
    ).astype(bf)

    box = np.asarray(inputs["box_data"], f32)
    fpks = []
    for c in range(NCORES):
        rawc = box[c * BPC:(c + 1) * BPC].reshape(BPC, T * 6, 6)
        rawc = rawc.reshape(BPC, 8, JB, 6).reshape(128, 900)
        fpks.append(np.ascontiguousarray(rawc, dtype=f32))
    return fpks, bpk


def _fast_path_ok(inputs):
    try:
        shapes = {
            "box_data": (B, T, 6, 6), "cat_table": (3, D), "geom_w1": (10, DH),
            "geom_b1": (DH,), "ln_g": (DH,), "ln_b": (DH,), "geom_w2": (DH, D),
            "geom_b2": (D,), "conf_w": (1, D), "conf_b": (D,),
            "center_w": (2, D), "center_b": (D,), "missing_emb": (1, D),
            "dist_w": (1, D), "dist_b": (D,), "cam_table": (NCAM, D),
        }
        for k, s in shapes.items():
            if tuple(np.asarray(inputs[k]).shape) != s:
                return False
        if not np.all(np.asarray(inputs["geom_b1"]) == 0):
            return False
        if not np.all(np.asarray(inputs["ln_g"]) == 1):
            return False
        if not np.all(np.asarray(inputs["ln_b"]) == 0):
            return False
        return True
    except Exception:
        return False


def _numpy_fallback(inputs):
    # Exact (slow) port of the reference for unexpected inputs.
    import math
    f32 = np.float32
    inp = {k: np.asarray(v) for k, v in inputs.items()}
    coords = inp["box_data"][..., :4].astype(f32)
    category = inp["box_data"][..., 4].astype(np.int32)
    conf = inp["box_data"][..., 5].astype(f32)
    norm = np.array([IW, IH, IW, IH], f32)
    cn = (coords / norm).reshape(B, T, NCAM, NB, 4)
    category = category.reshape(B, T, NCAM, NB)
    conf = conf.reshape(B, T, NCAM, NB, 1)
    presence = (cn.sum(-1) != 0).astype(f32)
    sort_key = category.astype(f32) + (1.0 - presence) * 1000.0
    idx = np.argsort(sort_key, axis=-1, kind="stable")
    cn = np.take_along_axis(cn, idx[..., None], axis=-2)
    category = np.take_along_axis(category, idx, axis=-1)
    conf = np.take_along_axis(conf, idx[..., None], axis=-2)
    presence = (cn.sum(-1) != 0).astype(f32)[..., None]
    x1, y1, x2, y2 = cn[..., 0], cn[..., 1], cn[..., 2], cn[..., 3]
    w, h = x2 - x1, y2 - y1
    cx, cy = (x1 + x2) * 0.5, (y1 + y2) * 0.5
    area, aspect = w * h, w / (h + 1e-6)
    dx, dy = cx[..., 0] - cx[..., 1], cy[..., 0] - cy[..., 1]
    dist = np.sqrt(dx * dx + dy * dy)[..., None]
    dist_tok = dist @ inp["dist_w"].astype(f32) + inp["dist_b"].astype(f32)
    geom = np.stack([x1, y1, x2, y2, w, h, cx, cy, area, aspect], axis=-1)
    z = geom @ inp["geom_w1"].astype(f32) + inp["geom_b1"].astype(f32)
    mu = z.mean(-1, keepdims=True)
    var = ((z - mu) ** 2).mean(-1, keepdims=True)
    xh = (z - mu) / np.sqrt(var + 1e-5) * inp["ln_g"].astype(f32) + inp["ln_b"].astype(f32)
    try:
        from scipy.special import erf as _erf
        g = xh * 0.5 * (1.0 + _erf(xh / np.sqrt(2.0)))
    except Exception:
        verf = np.vectorize(math.erf)
        g = xh * 0.5 * (1.0 + verf(xh / np.sqrt(2.0)))
    geom_p = g @ inp["geom_w2"].astype(f32) + inp["geom_b2"].astype(f32)
    cat_emb = inp["cat_table"].astype(f32)[category]
    conf_p = conf @ inp["conf_w"].astype(f32) + inp["conf_b"].astype(f32)
    center_p = np.stack([cx, cy], axis=-1) @ inp["center_w"].astype(f32) + inp["center_b"].astype(f32)
    cam_emb = inp["cam_table"].astype(f32).reshape(1, 1, NCAM, 1, D)
    tok = (geom_p + cat_emb + conf_p + center_p + cam_emb) * float(inp["scale"])
    tok = np.where(presence == 0, inp["missing_emb"].astype(f32)[0], tok)
    out = np.concatenate([dist_tok.reshape(B, T * NCAM, D),
                          tok.reshape(B, T * NCAM * NB, D)], axis=1)
    return out.astype(np.float32)


def _run(inputs, trace=False, tmpdir=None):
    from concourse.bass_utils import run_bass_kernel_spmd

    if "nc" not in _CACHE:
        _CACHE["nc"] = _build_nc()
    nc = _CACHE["nc"]

    fpks, bpk = _prep_inputs(inputs)
    in_maps = [{"fpk": fpks[c], "bpk": bpk} for c in range(NCORES)]
    res = run_bass_kernel_spmd(nc, in_maps, list(range(NCORES)),
                               trace=trace, tmpdir=tmpdir)
    out = np.concatenate(
        [np.concatenate([np.asarray(res.results[c]["outd"]),
                         np.asarray(res.results[c]["outb"])], axis=1)
         for c in range(NCORES)], axis=0)
    return out.astype(np.float32, copy=False), res


def kernel(**inputs):
    if not _fast_path_ok(inputs):
        return _numpy_fallback(inputs)
    out, _ = _run(inputs)
    return out


if __name__ == "__main__":
    import reference as ref
    inputs = {k: np.asarray(v) for k, v in ref.setup_inputs().items()}
    got = kernel(**inputs)
    exp = np.load("/tmp/expected.npy")
    d = got - exp
    print("rel fro:", np.linalg.norm(d) / np.linalg.norm(exp))
    print("absmax rel:", np.abs(d).max() / np.abs(exp).max())


# revision 36
# speedup vs baseline: 1.0034x; 1.0034x over previous
"""Trainium2 Bass kernel for nn_BoxEncoder (B=128, T=200, NC=3, NB=2, D=512, DH=256).

Strategy (data-parallel over batch, 16 batch items per core x 8 cores):

 - Token layout per core: partition p = bt*8 + q; j-slots 0..149 are box
   tokens (output rows 600 + q*150 + j), slots 150..224 are dist tokens
   (output rows q*75 + (j-150)).
 - All per-box scalars live as 32 feature columns per j-slot in a bf16
   T_feat tile [128, 225*32]; PE transposes of [128,128] chunks give
   feature-major lhsT blocks (cta).
 - LayerNorm stats WITHOUT materializing z: with L = chol(W1p@W1p.T/256)
   and m = W1p.sum(1)/256, one K=128 matmul per chunk against a
   block-diagonal [L|m] rhs yields u (10 cols) + mu per token;
   var = sum(u^2) - mu^2. DVE square+reduce finishes the stats.
 - rstd is folded into scaled feature columns f18..f27 (= geom * rstd)
   plus f28 = -mu*rstd, re-transposed (T2, box chunks only) into cta2.
 - h^T is computed weight-stationary: z_n^T = W1n^T @ x_n^T with N=512
   token columns per matmul, exact GELU applied straight out of PSUM into
   a persistent bf16 hT buffer [256(dh) x 19456(tok)] - no per-token
   transposes and no second z pass.
 - Per box token tile: out = hT0^T@W2hi + hT1^T@W2lo + raw_feats@W2x[cam]
   (K=32 extras fold cat one-hots, conf, center, b2+cam (presence-gated),
   missing_emb ((1-presence)-gated)). Missing boxes produce exactly
   missing_emb (their geometry path is gelu(0)=0).
 - dist tokens are extras-only matmuls (N=512, K=32).
 - Output staged to SBUF as bf16 (host upcasts to f32) halving HBM
   traffic; staging copies rotate across DVE/ACT/GPSIMD.
 - Phases are ordered to keep the tensor engine continuously busy so it
   ramps to its max p-state: T1 -> stats -> dist matmuls (while DVE does
   the stats postprocessing) -> T2 -> interleaved [main(b-1) | z_n^T(b)]
   blocks of 16 slots.
"""

import numpy as np
import ml_dtypes

B, T, NCAM, NB, D, DH = 128, 200, 3, 2, 512, 256
IW, IH = 640.0, 400.0
NCORES = 8
BPC = B // NCORES            # batch items per core
JB, JD = 150, 75             # box / dist j-slots per partition
J = JB + JD                  # 225
F = 32                       # feature columns per j-slot
NCH = (J * F + 127) // 128   # 57 transpose chunks (56 full + 1 of 32 cols)
NCH2 = 38                    # chunks re-transposed for the scaled features
NSLOT = NCH2 * 4             # 152 slots covered by stats / hT (150 box + 2)

_CACHE = {}


def _build_nc():
    from contextlib import ExitStack
    import concourse.bacc as bacc
    import concourse.mybir as mybir
    import concourse.tile as tile

    f32 = mybir.dt.float32
    bf16 = mybir.dt.bfloat16
    A = mybir.AluOpType
    AF = mybir.ActivationFunctionType
    AX = mybir.AxisListType

    # bf16 pack column offsets
    C_ID = 0
    C_W1N = 128                   # [128, 256] W1n tiled 4x (dh0 | dh1)
    C_W2HI = C_W1N + 256
    C_W2LO = C_W2HI + 512
    C_W2X = C_W2LO + 512          # 3 cam variants, 512 each
    C_LM = C_W2X + 3 * 512        # block-diag [L|m], 44 cols
    NBF = C_LM + 44

    nc = bacc.Bacc("TRN2", target_bir_lowering=False, debug=False,
                   num_devices=NCORES)
    fpk = nc.declare_dram_parameter("fpk", [128, 900], f32, isOutput=False)
    bpk = nc.declare_dram_parameter("bpk", [128, NBF], bf16, isOutput=False)
    outd = nc.declare_dram_parameter("outd", [BPC, 600, D], bf16, isOutput=True)
    outb = nc.declare_dram_parameter("outb", [BPC, 1200, D], bf16, isOutput=True)

    with ExitStack() as ctx:
        tc = ctx.enter_context(tile.TileContext(nc))
        cp = ctx.enter_context(tc.tile_pool(name="const", bufs=1))
        sc = ctx.enter_context(tc.tile_pool(name="scratch", bufs=1))
        # PSUM is bank-granular (8 banks). znp(3) spans all phases; each
        # phase scope adds its own pool (ctp 2 / ope 5 / op 5 <= 5 banks).
        znp = ctx.enter_context(tc.tile_pool(name="znps", bufs=3, space="PSUM"))
        bstg = ctx.enter_context(tc.tile_pool(name="bstage", bufs=3))
        dstg = ctx.enter_context(tc.tile_pool(name="dstage", bufs=3))

        fpack = cp.tile([128, 900], f32)
        nc.sync.dma_start(fpack[:], fpk[:])
        bpack = cp.tile([128, NBF], bf16)
        nc.sync.dma_start(bpack[:], bpk[:])

        raw = fpack[:, 0:900]
        idb = bpack[:, C_ID:C_ID + 128]
        w1n = bpack[:, C_W1N:C_W1N + 256]
        w2hi = bpack[:, C_W2HI:C_W2HI + 512]
        w2lo = bpack[:, C_W2LO:C_W2LO + 512]
        w2x = [bpack[:, C_W2X + c * 512: C_W2X + (c + 1) * 512] for c in range(3)]
        lm = bpack[:, C_LM:C_LM + 44]

        TF = cp.tile([128, J * F], bf16)
        nc.gpsimd.memset(TF[:], 0.0)

        TFj = TF.rearrange("p (j f) -> p j f", f=F)
        TFb = TFj[:, :JB, :]                       # box slots
        TFd = TFj[:, JB:, :]                       # dist slots
        TFbp = TF[:, :JB * F].rearrange("p (m g f) -> p m g f", g=2, f=F)
        raw6 = raw.rearrange("p (b s) -> p b s", s=6)
        rawp = raw.rearrange("p (m g s) -> p m g s", g=2, s=6)

        # ---------------- P1: feature planes (DVE+GpSimd, f32 scratch) --------
        # Built in two slot-halves so T1 transposes of the first 19 chunks
        # overlap with feature prep for the second half.
        sPres = sc.tile([128, JB], f32)
        sKey = sc.tile([128, JB], f32)
        sSwap = sc.tile([128, JD], f32)
        sD = sc.tile([128, JD], f32)
        sSD = sc.tile([128, JD], f32)
        sDg = sc.tile([128, JD], f32)
        sSDg = sc.tile([128, JD], f32)
        sw = [sc.tile([128, JB], f32, tag=f"swp{i}", name=f"swp{i}")
              for i in range(6)]
        sT0 = sc.tile([128, JB], f32)
        sT1 = sc.tile([128, JB], f32)
        sWn = sc.tile([128, JB], f32)
        sHn = sc.tile([128, JB], f32)
        sDx = sc.tile([128, JD], f32)
        sDy = sc.tile([128, JD], f32)

        def p1_half(s0, s1, p0, p1):
            r6 = raw6[:, s0:s1, :]
            rp = rawp[:, p0:p1, :, :]
            tb = TFb[:, s0:s1, :]
            tbp = TFbp[:, p0:p1, :, :]
            pres, key = sPres[:, s0:s1], sKey[:, s0:s1]
            swp, dd, sd_ = sSwap[:, p0:p1], sD[:, p0:p1], sSD[:, p0:p1]
            ddg, sdg = sDg[:, p0:p1], sSDg[:, p0:p1]
            t0, t1 = sT0[:, s0:s1], sT1[:, s0:s1]
            wn, hn = sWn[:, s0:s1], sHn[:, s0:s1]
            nc.vector.tensor_tensor(t0, r6[:, :, 0], r6[:, :, 1], A.add)
            nc.vector.tensor_tensor(t1, r6[:, :, 2], r6[:, :, 3], A.add)
            nc.vector.tensor_tensor(t0, t0, t1, A.add)
            nc.vector.tensor_scalar(pres, t0, 0.0, None, A.not_equal)
            # key = cat - 1000*pres (order-equiv. to cat + 1000*(1-pres))
            nc.vector.scalar_tensor_tensor(key, pres, -1000.0,
                                           r6[:, :, 4], A.mult, A.add)
            keyp = key.rearrange("p (m g) -> p m g", g=2)
            nc.vector.tensor_tensor(swp, keyp[:, :, 1], keyp[:, :, 0], A.is_lt)
            # compare-and-swap the 6 raw components + presence
            # (even components on DVE, odd on GpSimd, separate scratch)
            for i in range(6):
                ve, vo = rp[:, :, 0, i], rp[:, :, 1, i]
                dst = sw[i][:, s0:s1].rearrange("p (m g) -> p m g", g=2)
                if i % 2 == 0:
                    e, eD, eSD = nc.vector, dd, sd_
                else:
                    e, eD, eSD = nc.gpsimd, ddg, sdg
                e.tensor_tensor(eD, vo, ve, A.subtract)
                e.tensor_tensor(eSD, eD, swp, A.mult)
                e.tensor_tensor(dst[:, :, 0], ve, eSD, A.add)
                e.tensor_tensor(dst[:, :, 1], vo, eSD, A.subtract)
            presp = pres.rearrange("p (m g) -> p m g", g=2)
            nc.vector.tensor_tensor(dd, presp[:, :, 1], presp[:, :, 0],
                                    A.subtract)
            nc.vector.tensor_tensor(sd_, dd, swp, A.mult)
            nc.vector.tensor_tensor(tbp[:, :, 0, 14], presp[:, :, 0], sd_, A.add)
            nc.vector.tensor_tensor(tbp[:, :, 1, 14], presp[:, :, 1], sd_,
                                    A.subtract)
            sX1, sY1, sX2, sY2, sCat, sConf = [t[:, s0:s1] for t in sw]
            # f0..f3: normalized coords
            nc.vector.tensor_scalar(tb[:, :, 0], sX1, 1.0 / IW, None, A.mult)
            nc.vector.tensor_scalar(tb[:, :, 1], sY1, 1.0 / IH, None, A.mult)
            nc.vector.tensor_scalar(tb[:, :, 2], sX2, 1.0 / IW, None, A.mult)
            nc.vector.tensor_scalar(tb[:, :, 3], sY2, 1.0 / IH, None, A.mult)
            # f4 w, f5 h, f6 cx*2, f7 cy*2 (0.5 folded into the weights);
            # w/h/area/aspect in f32 scratch: the aspect denominator h+1e-6
            # would flip sign under bf16 rounding of h near -1e-6.
            nc.vector.tensor_tensor(wn, sX2, sX1, A.subtract)
            nc.vector.tensor_scalar(wn, wn, 1.0 / IW, None, A.mult)
            nc.vector.tensor_tensor(hn, sY2, sY1, A.subtract)
            nc.vector.tensor_scalar(hn, hn, 1.0 / IH, None, A.mult)
            nc.gpsimd.tensor_copy(tb[:, :, 4], wn)
            nc.gpsimd.tensor_copy(tb[:, :, 5], hn)
            nc.gpsimd.tensor_tensor(tb[:, :, 6], tb[:, :, 0], tb[:, :, 2], A.add)
            nc.gpsimd.tensor_tensor(tb[:, :, 7], tb[:, :, 1], tb[:, :, 3], A.add)
            # f8 area, f9 aspect = w / (h + 1e-6)
            nc.vector.tensor_tensor(tb[:, :, 8], wn, hn, A.mult)
            nc.vector.tensor_scalar(t0, hn, 1e-6, None, A.add)
            nc.vector.reciprocal(t1, t0)
            nc.vector.tensor_tensor(tb[:, :, 9], wn, t1, A.mult)
            # f10..12 cat one-hots * pres ; f13 conf*pres ; f15 = 1-pres
            for k in range(3):
                nc.vector.scalar_tensor_tensor(tb[:, :, 10 + k], sCat, float(k),
                                               tb[:, :, 14], A.is_equal, A.mult)
            nc.gpsimd.tensor_tensor(tb[:, :, 13], sConf, tb[:, :, 14], A.mult)
            nc.vector.tensor_scalar(tb[:, :, 15], tb[:, :, 14], -1.0, 1.0,
                                    A.mult, A.add)
            # dist tokens: f16 = 0.5*sqrt(dx2^2+dy2^2) (cx doubled), f17 = 1
            dx, dy = sDx[:, p0:p1], sDy[:, p0:p1]
            nc.vector.tensor_tensor(dx, tbp[:, :, 0, 6], tbp[:, :, 1, 6],
                                    A.subtract)
            nc.vector.tensor_tensor(dy, tbp[:, :, 0, 7], tbp[:, :, 1, 7],
                                    A.subtract)
            nc.vector.tensor_tensor(dx, dx, dx, A.mult)
            nc.vector.tensor_tensor(dy, dy, dy, A.mult)
            nc.vector.tensor_tensor(dx, dx, dy, A.add)
            nc.scalar.activation(TFd[:, p0:p1, 16], dx, AF.Sqrt, scale=0.25)
            nc.vector.memset(TFd[:, p0:p1, 17], 1.0)

        hT = [cp.tile([128, NSLOT * 128], bf16, tag=f"hT{i}", name=f"hT{i}")
              for i in range(2)]
        hT4 = [h.rearrange("p (s4 jj q) -> p s4 jj q", jj=4, q=128) for h in hT]
        cta = cp.tile([128, NCH * 128], bf16)
        cta2 = cp.tile([128, NCH2 * 128], bf16)
        vd = outd.rearrange("b (q r) d -> b q r d", q=8)
        vb = outb.rearrange("b (q r) d -> b q r d", q=8)
        NBLK = (NSLOT + 15) // 16          # 10 blocks; block 9 is half-size

        def zn_mm(b, k):
            # z_n^T matmul k (dhc=k//4, jj=k%4) of slot-block b + exact GELU
            dhc, jj = k // 4, k % 4
            c0 = b * 512
            nb = min(512, NCH2 * 128 - c0)
            tcnt = nb // 128
            zt = znp.tile([128, 512], f32, tag="zn")
            nc.tensor.matmul(
                zt[:, :nb],
                w1n[32 * jj:32 * jj + 32, dhc * 128:(dhc + 1) * 128],
                cta2[32 * jj:32 * jj + 32, c0:c0 + nb],
                start=True, stop=True, tile_position=(32 * jj, 0))
            ztv = zt.rearrange("p (t q) -> p t q", q=128)
            nc.scalar.activation(hT4[dhc][:, 4 * b:4 * b + tcnt, jj, :],
                                 ztv[:, 0:tcnt, :], AF.Gelu)

        with tc.tile_pool(name="ctps", bufs=2, space="PSUM") as ctp:
            # -------- P1 halves + T1 transposes (segment A overlaps half 2) --
            def t1_seg(c0, c1):
                for ci in range(c0, c1):
                    w_cols = min(128, J * F - ci * 128)
                    ps = ctp.tile([128, 128], bf16, tag="ct")
                    nc.tensor.transpose(ps[:w_cols, :],
                                        TF[:, ci * 128: ci * 128 + w_cols], idb)
                    dst = cta[:w_cols, ci * 128: ci * 128 + 128]
                    if ci % 3 == 2:
                        nc.scalar.copy(dst, ps[:w_cols, :])
                    else:
                        nc.vector.tensor_copy(dst, ps[:w_cols, :])

            p1_half(0, 76, 0, 38)
            t1_seg(0, 19)
            p1_half(76, JB, 38, JD)
            t1_seg(19, NCH)

        with tc.tile_pool(name="opse", bufs=5, space="PSUM") as ope:
            # -------- stats: u/mu per token via [L|m] matmuls ---------------
            stats_sb = sc.tile([128, NCH2 * 44], f32)
            for ci in range(NCH2):
                sm = ope.tile([128, D], f32, tag="o")
                nc.tensor.matmul(sm[:, 0:44], cta[:, ci * 128:(ci + 1) * 128],
                                 lm, start=True, stop=True)
                nc.vector.tensor_copy(stats_sb[:, ci * 44:(ci + 1) * 44],
                                      sm[:, 0:44])

            # DVE post: var = sum(u^2) - mu^2 ; rstd = 1/sqrt(var+eps)
            sq = sc.tile([128, NCH2 * 44], f32)
            nc.vector.tensor_tensor(sq[:], stats_sb[:], stats_sb[:], A.mult)
            sqv = sq.rearrange("p (s e) -> p s e", e=11)
            stv = stats_sb.rearrange("p (s e) -> p s e", e=11)
            usum = sc.tile([128, NSLOT], f32)
            nc.vector.tensor_reduce(usum[:], sqv[:, :, 0:10], AX.X, A.add)
            var = sc.tile([128, NSLOT], f32)
            nc.vector.tensor_tensor(var[:], usum[:], sqv[:, :, 10], A.subtract)
            eps = sc.tile([128, 1], f32)
            nc.vector.memset(eps[:], 1e-5)
            sd = sc.tile([128, NSLOT], f32)
            nc.scalar.activation(sd[:], var[:], AF.Sqrt, bias=eps[:])
            rstd = sc.tile([128, NSLOT], f32)
            nc.vector.reciprocal(rstd[:], sd[:])
            # scaled features f18..f27 = geom * rstd ; f28 = -mu*rstd
            for i in range(10):
                eng_i = nc.vector if i % 2 == 0 else nc.gpsimd
                eng_i.tensor_tensor(TFb[:, :, 18 + i], TFb[:, :, i],
                                    rstd[:, 0:JB], A.mult)
            nc.vector.scalar_tensor_tensor(TFb[:, :, 28], stv[:, 0:JB, 10],
                                           -1.0, rstd[:, 0:JB], A.mult, A.mult)

            # -------- dist tokens (deep ope pool hides the staging casts) ---
            dist_stage = None
            for dk in range(JD):
                j = JB + dk
                ci, jj = j // 4, j % 4
                o = ope.tile([128, D], f32, tag="o")
                nc.tensor.matmul(o[:],
                                 cta[32 * jj:32 * jj + 32,
                                     ci * 128:(ci + 1) * 128],
                                 w2x[0][32 * jj:32 * jj + 32, :],
                                 start=True, stop=True,
                                 tile_position=(32 * jj, 0))
                if dist_stage is None:
                    dist_stage = dstg.tile([128, 8 * D], bf16, tag="dstage")
                slot = dk % 8
                dst = dist_stage[:, slot * D:(slot + 1) * D]
                if dk % 2 == 0:
                    nc.scalar.copy(dst, o[:])
                else:
                    nc.vector.tensor_copy(dst, o[:])
                if slot == 7 or dk == JD - 1:
                    g = slot + 1
                    nc.sync.dma_start(vd[:, :, dk - g + 1: dk + 1, :],
                                      dist_stage[:, : g * D])
                    dist_stage = None

        with tc.tile_pool(name="ctps2", bufs=2, space="PSUM") as ctp:
            # -------- T2: re-transpose box chunks; weave z_n block 0 in ----
            for ci in range(NCH2):
                ps = ctp.tile([128, 128], bf16, tag="ct")
                nc.tensor.transpose(ps[:], TF[:, ci * 128:(ci + 1) * 128], idb)
                dst = cta2[:, ci * 128:(ci + 1) * 128]
                if ci % 3 == 2:
                    nc.scalar.copy(dst, ps[:])
                else:
                    nc.vector.tensor_copy(dst, ps[:])
                if 4 <= ci < 12:
                    zn_mm(0, ci - 4)

        # -------- main box loop; z_n of block b+1 woven into block b --------
        # Three slots' accumulation groups are interleaved so consecutive PE
        # instructions hit different PSUM banks and pipeline-overlap.
        with tc.tile_pool(name="ops", bufs=5, space="PSUM") as op:
            box_stage = None
            gstart = 0
            eng = 0
            for j0 in range(0, JB, 3):
                grp = (j0, j0 + 1, j0 + 2)
                ot = {j: op.tile([128, D], f32, tag="o", name=f"o{j}")
                      for j in grp}
                for j in grp:
                    nc.tensor.matmul(ot[j][:], hT[0][:, j * 128:(j + 1) * 128],
                                     w2hi, start=True, stop=False)
                for j in grp:
                    nc.tensor.matmul(ot[j][:], hT[1][:, j * 128:(j + 1) * 128],
                                     w2lo, start=False, stop=False)
                for j in grp:
                    ci, jj = j // 4, j % 4
                    cam = (j % 6) // 2
                    nc.tensor.matmul(ot[j][:], cta[32 * jj:32 * jj + 32,
                                               ci * 128:(ci + 1) * 128],
                                     w2x[cam][32 * jj:32 * jj + 32, :],
                                     start=False, stop=True,
                                     tile_position=(32 * jj, 0))
                for j in grp:
                    if j % 2 == 0 and j // 16 + 1 < NBLK:
                        zn_mm(j // 16 + 1, (j % 16) // 2)
                for j in grp:
                    if box_stage is None:
                        box_stage = bstg.tile([128, 8 * D], bf16, tag="bstage")
                        gstart = j
                    off = j - gstart
                    dst = box_stage[:, off * D:(off + 1) * D]
                    if eng == 0:
                        nc.vector.tensor_copy(dst, ot[j][:])
                    else:
                        nc.scalar.copy(dst, ot[j][:])
                    eng = (eng + 1) % 2
                    if off == 7 or j == JB - 1 or (j >= 144 and off >= 2):
                        nc.sync.dma_start(vb[:, :, gstart: j + 1, :],
                                          box_stage[:, : (off + 1) * D])
                        box_stage = None

    nc.compile()
    return nc


def _prep_inputs(inputs):
    f32 = np.float32
    bf = ml_dtypes.bfloat16
    scale = float(np.asarray(inputs["scale"]))

    # W1 with the cx/cy doubling folded in (features store 2*cx, 2*cy)
    W1p = np.asarray(inputs["geom_w1"], f32).copy()
    W1p[6] *= 0.5
    W1p[7] *= 0.5

    # W1n: rows f18..f27 -> W1p, f28 -> ones (the -mu*rstd passthrough)
    W1n = np.zeros((32, DH), f32)
    W1n[18:28] = W1p
    W1n[28] = 1.0
    w1n_cols = np.concatenate([W1n[:, :128], W1n[:, 128:]], axis=1)  # [32, 256]
    w1n_rep = np.tile(w1n_cols, (4, 1))

    W2s = scale * np.asarray(inputs["geom_w2"], f32)
    w2hi, w2lo = W2s[:128], W2s[128:]

    cat_t = np.asarray(inputs["cat_table"], f32)
    cam_t = np.asarray(inputs["cam_table"], f32)
    bias_row = (np.asarray(inputs["geom_b2"], f32)
                + np.asarray(inputs["conf_b"], f32)
                + np.asarray(inputs["center_b"], f32))
    w2x_reps = []
    for c in range(3):
        W2x = np.zeros((32, D), f32)
        W2x[6] = scale * np.asarray(inputs["center_w"], f32)[0] * 0.5
        W2x[7] = scale * np.asarray(inputs["center_w"], f32)[1] * 0.5
        W2x[10:13] = scale * cat_t
        W2x[13] = scale * np.asarray(inputs["conf_w"], f32)[0]
        W2x[14] = scale * (bias_row + cam_t[c])
        W2x[15] = np.asarray(inputs["missing_emb"], f32)[0]
        W2x[16] = np.asarray(inputs["dist_w"], f32)[0]
        W2x[17] = np.asarray(inputs["dist_b"], f32)
        w2x_reps.append(np.tile(W2x, (4, 1)))

    # stats: L = chol(W1p W1p^T / 256), m = W1p.sum(1)/256;
    # block-diag rhs [128, 44]: rows 32j'+f (f<10) -> cols 11j'+(L[f,:]|m[f])
    G = (W1p @ W1p.T) / float(DH)
    Lc = np.linalg.cholesky(G + 1e-12 * np.eye(10))
    m = W1p.sum(axis=1) / float(DH)
    LM = np.zeros((128, 44), f32)
    for jp in range(4):
        LM[32 * jp:32 * jp + 10, 11 * jp:11 * jp + 10] = Lc
        LM[32 * jp:32 * jp + 10, 11 * jp + 10] = m

    idf32 = np.eye(128, dtype=f32)
    bpk = np.concatenate(
        [idf32, w1n_rep, w2hi, w2lo] + w2x_reps + [LM], axis=1
    ).astype(bf)

    box = np.asarray(inputs["box_data"], f32)
    fpks = []
    for c in range(NCORES):
        rawc = box[c * BPC:(c + 1) * BPC].reshape(BPC, T * 6, 6)
        rawc = rawc.reshape(BPC, 8, JB, 6).reshape(128, 900)
        fpks.append(np.ascontiguousarray(rawc, dtype=f32))
    return fpks, bpk


def _fast_path_ok(inputs):
    try:
        shapes = {
            "box_data": (B, T, 6, 6), "cat_table": (3, D), "geom_w1": (10, DH),
            "geom_b1": (DH,), "ln_g": (DH,), "ln_b": (DH,), "geom_w2": (DH, D),
            "geom_b2": (D,), "conf_w": (1, D), "conf_b": (D,),
            "center_w": (2, D), "center_b": (D,), "missing_emb": (1, D),
            "dist_w": (1, D), "dist_b": (D,), "cam_table": (NCAM, D),
        }
        for k, s in shapes.items():
            if tuple(np.asarray(inputs[k]).shape) != s:
                return False
        if not np.all(np.asarray(inputs["geom_b1"]) == 0):
            return False
        if not np.all(np.asarray(inputs["ln_g"]) == 1):
            return False
        if not np.all(np.asarray(inputs["ln_b"]) == 0):
            return False
        return True
    except Exception:
        return False


def _numpy_fallback(inputs):
    # Exact (slow) port of the reference for unexpected inputs.
    import math
    f32 = np.float32
    inp = {k: np.asarray(v) for k, v in inputs.items()}
    coords = inp["box_data"][..., :4].astype(f32)
    category = inp["box_data"][..., 4].astype(np.int32)
    conf = inp["box_data"][..., 5].astype(f32)
    norm = np.array([IW, IH, IW, IH], f32)
    cn = (coords / norm).reshape(B, T, NCAM, NB, 4)
    category = category.reshape(B, T, NCAM, NB)
    conf = conf.reshape(B, T, NCAM, NB, 1)
    presence = (cn.sum(-1) != 0).astype(f32)
    sort_key = category.astype(f32) + (1.0 - presence) * 1000.0
    idx = np.argsort(sort_key, axis=-1, kind="stable")
    cn = np.take_along_axis(cn, idx[..., None], axis=-2)
    category = np.take_along_axis(category, idx, axis=-1)
    conf = np.take_along_axis(conf, idx[..., None], axis=-2)
    presence = (cn.sum(-1) != 0).astype(f32)[..., None]
    x1, y1, x2, y2 = cn[..., 0], cn[..., 1], cn[..., 2], cn[..., 3]
    w, h = x2 - x1, y2 - y1
    cx, cy = (x1 + x2) * 0.5, (y1 + y2) * 0.5
    area, aspect = w * h, w / (h + 1e-6)
    dx, dy = cx[..., 0] - cx[..., 1], cy[..., 0] - cy[..., 1]
    dist = np.sqrt(dx * dx + dy * dy)[..., None]
    dist_tok = dist @ inp["dist_w"].astype(f32) + inp["dist_b"].astype(f32)
    geom = np.stack([x1, y1, x2, y2, w, h, cx, cy, area, aspect], axis=-1)
    z = geom @ inp["geom_w1"].astype(f32) + inp["geom_b1"].astype(f32)
    mu = z.mean(-1, keepdims=True)
    var = ((z - mu) ** 2).mean(-1, keepdims=True)
    xh = (z - mu) / np.sqrt(var + 1e-5) * inp["ln_g"].astype(f32) + inp["ln_b"].astype(f32)
    try:
        from scipy.special import erf as _erf
        g = xh * 0.5 * (1.0 + _erf(xh / np.sqrt(2.0)))
    except Exception:
        verf = np.vectorize(math.erf)
        g = xh * 0.5 * (1.0 + verf(xh / np.sqrt(2.0)))
    geom_p = g @ inp["geom_w2"].astype(f32) + inp["geom_b2"].astype(f32)
    cat_emb = inp["cat_table"].astype(f32)[category]
    conf_p = conf @ inp["conf_w"].astype(f32) + inp["conf_b"].astype(f32)
    center_p = np.stack([cx, cy], axis=-1) @ inp["center_w"].astype(f32) + inp["center_b"].astype(f32)
    cam_emb = inp["cam_table"].astype(f32).reshape(1, 1, NCAM, 1, D)
    tok = (geom_p + cat_emb + conf_p + center_p + cam_emb) * float(inp["scale"])
    tok = np.where(presence == 0, inp["missing_emb"].astype(f32)[0], tok)
    out = np.concatenate([dist_tok.reshape(B, T * NCAM, D),
                          tok.reshape(B, T * NCAM * NB, D)], axis=1)
    return out.astype(np.float32)


def _run(inputs, trace=False, tmpdir=None):
    from concourse.bass_utils import run_bass_kernel_spmd

    if "nc" not in _CACHE:
        _CACHE["nc"] = _build_nc()
    nc = _CACHE["nc"]

    fpks, bpk = _prep_inputs(inputs)
    in_maps = [{"fpk": fpks[c], "bpk": bpk} for c in range(NCORES)]
    res = run_bass_kernel_spmd(nc, in_maps, list(range(NCORES)),
                               trace=trace, tmpdir=tmpdir)
    out = np.concatenate(
        [np.concatenate([np.asarray(res.results[c]["outd"]),
                         np.asarray(res.results[c]["outb"])], axis=1)
         for c in range(NCORES)], axis=0)
    return out.astype(np.float32, copy=False), res


def kernel(**inputs):
    if not _fast_path_ok(inputs):
        return _numpy_fallback(inputs)
    out, _ = _run(inputs)
    return out


if __name__ == "__main__":
    import reference as ref
    inputs = {k: np.asarray(v) for k, v in ref.setup_inputs().items()}
    got = kernel(**inputs)
    exp = np.load("/tmp/expected.npy")
    d = got - exp
    print("rel fro:", np.linalg.norm(d) / np.linalg.norm(exp))
    print("absmax rel:", np.abs(d).max() / np.abs(exp).max())


# revision 39
# speedup vs baseline: 1.0069x; 1.0035x over previous
"""Trainium2 Bass kernel for nn_BoxEncoder (B=128, T=200, NC=3, NB=2, D=512, DH=256).

Strategy (data-parallel over batch, 16 batch items per core x 8 cores):

 - Token layout per core: partition p = bt*8 + q; j-slots 0..149 are box
   tokens (output rows 600 + q*150 + j), slots 150..224 are dist tokens
   (output rows q*75 + (j-150)).
 - All per-box scalars live as 32 feature columns per j-slot in a bf16
   T_feat tile [128, 225*32]; PE transposes of [128,128] chunks give
   feature-major lhsT blocks (cta).
 - LayerNorm stats WITHOUT materializing z: with L = chol(W1p@W1p.T/256)
   and m = W1p.sum(1)/256, one K=128 matmul per chunk against a
   block-diagonal [L|m] rhs yields u (10 cols) + mu per token;
   var = sum(u^2) - mu^2. DVE square+reduce finishes the stats.
 - rstd is folded into scaled feature columns f18..f27 (= geom * rstd)
   plus f28 = -mu*rstd, re-transposed (T2, box chunks only) into cta2.
 - h^T is computed weight-stationary: z_n^T = W1n^T @ x_n^T with N=512
   token columns per matmul, exact GELU applied straight out of PSUM into
   a persistent bf16 hT buffer [256(dh) x 19456(tok)] - no per-token
   transposes and no second z pass.
 - Per box token tile: out = hT0^T@W2hi + hT1^T@W2lo + raw_feats@W2x[cam]
   (K=32 extras fold cat one-hots, conf, center, b2+cam (presence-gated),
   missing_emb ((1-presence)-gated)). Missing boxes produce exactly
   missing_emb (their geometry path is gelu(0)=0).
 - dist tokens are extras-only matmuls (N=512, K=32).
 - Output staged to SBUF as bf16 (host upcasts to f32) halving HBM
   traffic; staging copies rotate across DVE/ACT/GPSIMD.
 - Phases are ordered to keep the tensor engine continuously busy so it
   ramps to its max p-state: T1 -> stats -> dist matmuls (while DVE does
   the stats postprocessing) -> T2 -> interleaved [main(b-1) | z_n^T(b)]
   blocks of 16 slots.
"""

import numpy as np
import ml_dtypes

B, T, NCAM, NB, D, DH = 128, 200, 3, 2, 512, 256
IW, IH = 640.0, 400.0
NCORES = 8
BPC = B // NCORES            # batch items per core
JB, JD = 150, 75             # box / dist j-slots per partition
J = JB + JD                  # 225
F = 32                       # feature columns per j-slot
NCH = (J * F + 127) // 128   # 57 transpose chunks (56 full + 1 of 32 cols)
NCH2 = 38                    # chunks re-transposed for the scaled features
NSLOT = NCH2 * 4             # 152 slots covered by stats / hT (150 box + 2)

_CACHE = {}


def _build_nc():
    from contextlib import ExitStack
    import concourse.bacc as bacc
    import concourse.mybir as mybir
    import concourse.tile as tile

    f32 = mybir.dt.float32
    bf16 = mybir.dt.bfloat16
    A = mybir.AluOpType
    AF = mybir.ActivationFunctionType
    AX = mybir.AxisListType

    # bf16 pack column offsets
    C_ID = 0
    C_W1N = 128                   # [128, 256] W1n tiled 4x (dh0 | dh1)
    C_W2HI = C_W1N + 256
    C_W2LO = C_W2HI + 512
    C_W2X = C_W2LO + 512          # 3 cam variants, 512 each
    C_LM = C_W2X + 3 * 512        # block-diag [L|m], 44 cols
    NBF = C_LM + 44

    nc = bacc.Bacc("TRN2", target_bir_lowering=False, debug=False,
                   num_devices=NCORES)
    fpk = nc.declare_dram_parameter("fpk", [128, 900], f32, isOutput=False)
    bpk = nc.declare_dram_parameter("bpk", [128, NBF], bf16, isOutput=False)
    outd = nc.declare_dram_parameter("outd", [BPC, 600, D], bf16, isOutput=True)
    outb = nc.declare_dram_parameter("outb", [BPC, 1200, D], bf16, isOutput=True)

    with ExitStack() as ctx:
        tc = ctx.enter_context(tile.TileContext(nc))
        cp = ctx.enter_context(tc.tile_pool(name="const", bufs=1))
        sc = ctx.enter_context(tc.tile_pool(name="scratch", bufs=1))
        # PSUM is bank-granular (8 banks). znp(3) spans all phases; each
        # phase scope adds its own pool (ctp 2 / ope 5 / op 5 <= 5 banks).
        znp = ctx.enter_context(tc.tile_pool(name="znps", bufs=3, space="PSUM"))
        bstg = ctx.enter_context(tc.tile_pool(name="bstage", bufs=3))
        dstg = ctx.enter_context(tc.tile_pool(name="dstage", bufs=3))

        fpack = cp.tile([128, 900], f32)
        nc.sync.dma_start(fpack[:], fpk[:])
        bpack = cp.tile([128, NBF], bf16)
        nc.sync.dma_start(bpack[:], bpk[:])

        raw = fpack[:, 0:900]
        idb = bpack[:, C_ID:C_ID + 128]
        w1n = bpack[:, C_W1N:C_W1N + 256]
        w2hi = bpack[:, C_W2HI:C_W2HI + 512]
        w2lo = bpack[:, C_W2LO:C_W2LO + 512]
        w2x = [bpack[:, C_W2X + c * 512: C_W2X + (c + 1) * 512] for c in range(3)]
        lm = bpack[:, C_LM:C_LM + 44]

        TF = cp.tile([128, J * F], bf16)
        nc.gpsimd.memset(TF[:], 0.0)

        TFj = TF.rearrange("p (j f) -> p j f", f=F)
        TFb = TFj[:, :JB, :]                       # box slots
        TFd = TFj[:, JB:, :]                       # dist slots
        TFbp = TF[:, :JB * F].rearrange("p (m g f) -> p m g f", g=2, f=F)
        raw6 = raw.rearrange("p (b s) -> p b s", s=6)
        rawp = raw.rearrange("p (m g s) -> p m g s", g=2, s=6)

        # ---------------- P1: feature planes (DVE, f32 scratch) ----------------
        sPres = sc.tile([128, JB], f32)
        sKey = sc.tile([128, JB], f32)
        sSwap = sc.tile([128, JD], f32)
        sD = sc.tile([128, JD], f32)
        sSD = sc.tile([128, JD], f32)
        sw = [sc.tile([128, JB], f32, tag=f"swp{i}", name=f"swp{i}")
              for i in range(6)]
        sT0 = sc.tile([128, JB], f32)
        sT1 = sc.tile([128, JB], f32)

        nc.vector.tensor_tensor(sT0[:], raw6[:, :, 0], raw6[:, :, 1], A.add)
        nc.vector.tensor_tensor(sT1[:], raw6[:, :, 2], raw6[:, :, 3], A.add)
        nc.vector.tensor_tensor(sT0[:], sT0[:], sT1[:], A.add)
        nc.vector.tensor_scalar(sPres[:], sT0[:], 0.0, None, A.not_equal)
        # key = cat - 1000*pres  (order-equivalent to cat + 1000*(1-pres))
        nc.vector.scalar_tensor_tensor(sKey[:], sPres[:], -1000.0,
                                       raw6[:, :, 4], A.mult, A.add)
        sKeyp = sKey.rearrange("p (m g) -> p m g", g=2)
        nc.vector.tensor_tensor(sSwap[:], sKeyp[:, :, 1], sKeyp[:, :, 0], A.is_lt)

        # compare-and-swap each of the 6 raw components + presence
        # (even components on DVE, odd on GpSimd, with separate scratch)
        sDg = sc.tile([128, JD], f32)
        sSDg = sc.tile([128, JD], f32)
        for i in range(6):
            ve, vo = rawp[:, :, 0, i], rawp[:, :, 1, i]
            dst = sw[i].rearrange("p (m g) -> p m g", g=2)
            if i % 2 == 0:
                e, eD, eSD = nc.vector, sD, sSD
            else:
                e, eD, eSD = nc.gpsimd, sDg, sSDg
            e.tensor_tensor(eD[:], vo, ve, A.subtract)
            e.tensor_tensor(eSD[:], eD[:], sSwap[:], A.mult)
            e.tensor_tensor(dst[:, :, 0], ve, eSD[:], A.add)
            e.tensor_tensor(dst[:, :, 1], vo, eSD[:], A.subtract)
        sPresP = sPres.rearrange("p (m g) -> p m g", g=2)
        nc.vector.tensor_tensor(sD[:], sPresP[:, :, 1], sPresP[:, :, 0], A.subtract)
        nc.vector.tensor_tensor(sSD[:], sD[:], sSwap[:], A.mult)
        nc.vector.tensor_tensor(TFbp[:, :, 0, 14], sPresP[:, :, 0], sSD[:], A.add)
        nc.vector.tensor_tensor(TFbp[:, :, 1, 14], sPresP[:, :, 1], sSD[:], A.subtract)

        sX1, sY1, sX2, sY2, sCat, sConf = sw
        # f0..f3: normalized coords
        nc.vector.tensor_scalar(TFb[:, :, 0], sX1[:], 1.0 / IW, None, A.mult)
        nc.vector.tensor_scalar(TFb[:, :, 1], sY1[:], 1.0 / IH, None, A.mult)
        nc.vector.tensor_scalar(TFb[:, :, 2], sX2[:], 1.0 / IW, None, A.mult)
        nc.vector.tensor_scalar(TFb[:, :, 3], sY2[:], 1.0 / IH, None, A.mult)
        # f4 w, f5 h, f6 cx*2, f7 cy*2 (the 0.5 is folded into the weights)
        # w/h/area/aspect computed in f32 scratch: the aspect denominator
        # h+1e-6 would flip sign under bf16 rounding of h near -1e-6.
        sWn = sc.tile([128, JB], f32)
        sHn = sc.tile([128, JB], f32)
        nc.vector.tensor_tensor(sWn[:], sX2[:], sX1[:], A.subtract)
        nc.vector.tensor_scalar(sWn[:], sWn[:], 1.0 / IW, None, A.mult)
        nc.vector.tensor_tensor(sHn[:], sY2[:], sY1[:], A.subtract)
        nc.vector.tensor_scalar(sHn[:], sHn[:], 1.0 / IH, None, A.mult)
        nc.gpsimd.tensor_copy(TFb[:, :, 4], sWn[:])
        nc.gpsimd.tensor_copy(TFb[:, :, 5], sHn[:])
        nc.gpsimd.tensor_tensor(TFb[:, :, 6], TFb[:, :, 0], TFb[:, :, 2], A.add)
        nc.gpsimd.tensor_tensor(TFb[:, :, 7], TFb[:, :, 1], TFb[:, :, 3], A.add)
        # f8 area, f9 aspect = w / (h + 1e-6)
        nc.vector.tensor_tensor(TFb[:, :, 8], sWn[:], sHn[:], A.mult)
        sHp = sT0
        nc.vector.tensor_scalar(sHp[:], sHn[:], 1e-6, None, A.add)
        sR = sT1
        nc.vector.reciprocal(sR[:], sHp[:])
        nc.vector.tensor_tensor(TFb[:, :, 9], sWn[:], sR[:], A.mult)
        # f10..12 cat one-hots * pres ; f13 conf*pres ; f15 = 1-pres
        for k in range(3):
            nc.vector.scalar_tensor_tensor(TFb[:, :, 10 + k], sCat[:], float(k),
                                           TFb[:, :, 14], A.is_equal, A.mult)
        nc.gpsimd.tensor_tensor(TFb[:, :, 13], sConf[:], TFb[:, :, 14], A.mult)
        nc.vector.tensor_scalar(TFb[:, :, 15], TFb[:, :, 14], -1.0, 1.0,
                                A.mult, A.add)
        # dist tokens: f16 = 0.5*sqrt(dx2^2+dy2^2) (cx stored doubled), f17 = 1
        sDx = sc.tile([128, JD], f32)
        sDy = sc.tile([128, JD], f32)
        nc.vector.tensor_tensor(sDx[:], TFbp[:, :, 0, 6], TFbp[:, :, 1, 6], A.subtract)
        nc.vector.tensor_tensor(sDy[:], TFbp[:, :, 0, 7], TFbp[:, :, 1, 7], A.subtract)
        nc.vector.tensor_tensor(sDx[:], sDx[:], sDx[:], A.mult)
        nc.vector.tensor_tensor(sDy[:], sDy[:], sDy[:], A.mult)
        nc.vector.tensor_tensor(sDx[:], sDx[:], sDy[:], A.add)
        nc.scalar.activation(TFd[:, :, 16], sDx[:], AF.Sqrt, scale=0.25)
        nc.vector.memset(TFd[:, :, 17], 1.0)

        hT = [cp.tile([128, NSLOT * 128], bf16, tag=f"hT{i}", name=f"hT{i}")
              for i in range(2)]
        hT4 = [h.rearrange("p (s4 jj q) -> p s4 jj q", jj=4, q=128) for h in hT]
        cta = cp.tile([128, NCH * 128], bf16)
        cta2 = cp.tile([128, NCH2 * 128], bf16)
        vd = outd.rearrange("b (q r) d -> b q r d", q=8)
        vb = outb.rearrange("b (q r) d -> b q r d", q=8)
        NBLK = (NSLOT + 15) // 16          # 10 blocks; block 9 is half-size

        def zn_mm(b, k):
            # z_n^T matmul k (dhc=k//4, jj=k%4) of slot-block b + exact GELU
            dhc, jj = k // 4, k % 4
            c0 = b * 512
            nb = min(512, NCH2 * 128 - c0)
            tcnt = nb // 128
            zt = znp.tile([128, 512], f32, tag="zn")
            nc.tensor.matmul(
                zt[:, :nb],
                w1n[32 * jj:32 * jj + 32, dhc * 128:(dhc + 1) * 128],
                cta2[32 * jj:32 * jj + 32, c0:c0 + nb],
                start=True, stop=True, tile_position=(32 * jj, 0))
            ztv = zt.rearrange("p (t q) -> p t q", q=128)
            nc.scalar.activation(hT4[dhc][:, 4 * b:4 * b + tcnt, jj, :],
                                 ztv[:, 0:tcnt, :], AF.Gelu)

        with tc.tile_pool(name="ctps", bufs=2, space="PSUM") as ctp:
            # -------- T1: transpose T_feat chunks -> bf16 lhsT tiles --------
            for ci in range(NCH):
                w_cols = min(128, J * F - ci * 128)
                ps = ctp.tile([128, 128], bf16, tag="ct")
                nc.tensor.transpose(ps[:w_cols, :],
                                    TF[:, ci * 128: ci * 128 + w_cols], idb)
                dst = cta[:w_cols, ci * 128: ci * 128 + 128]
                if ci % 3 == 2:
                    nc.scalar.copy(dst, ps[:w_cols, :])
                else:
                    nc.vector.tensor_copy(dst, ps[:w_cols, :])

        with tc.tile_pool(name="opse", bufs=5, space="PSUM") as ope:
            # -------- stats: u/mu per token via [L|m] matmuls ---------------
            stats_sb = sc.tile([128, NCH2 * 44], f32)
            for ci in range(NCH2):
                sm = ope.tile([128, D], f32, tag="o")
                nc.tensor.matmul(sm[:, 0:44], cta[:, ci * 128:(ci + 1) * 128],
                                 lm, start=True, stop=True)
                nc.vector.tensor_copy(stats_sb[:, ci * 44:(ci + 1) * 44],
                                      sm[:, 0:44])

            # -------- dist tokens (deep ope pool hides the staging casts) ---
            dstate = {"stage": None}

            def dist_iter(dk):
                j = JB + dk
                ci, jj = j // 4, j % 4
                o = ope.tile([128, D], f32, tag="o", name="o")
                nc.tensor.matmul(o[:],
                                 cta[32 * jj:32 * jj + 32,
                                     ci * 128:(ci + 1) * 128],
                                 w2x[0][32 * jj:32 * jj + 32, :],
                                 start=True, stop=True,
                                 tile_position=(32 * jj, 0))
                if dstate["stage"] is None:
                    dstate["stage"] = dstg.tile([128, 8 * D], bf16,
                                                tag="dstage", name="dstage")
                slot = dk % 8
                dst = dstate["stage"][:, slot * D:(slot + 1) * D]
                if dk % 2 == 0:
                    nc.scalar.copy(dst, o[:])
                else:
                    nc.vector.tensor_copy(dst, o[:])
                if slot == 7 or dk == JD - 1:
                    g = slot + 1
                    nc.sync.dma_start(vd[:, :, dk - g + 1: dk + 1, :],
                                      dstate["stage"][:, : g * D])
                    dstate["stage"] = None

            # first 3 dist DMA groups go ahead of the DVE post chain so the
            # ope pool is never starved while DVE crunches the statistics
            for dk in range(24):
                dist_iter(dk)

            # DVE post: var = sum(u^2) - mu^2 ; rstd = 1/sqrt(var+eps)
            sq = sc.tile([128, NCH2 * 44], f32)
            nc.vector.tensor_tensor(sq[:], stats_sb[:], stats_sb[:], A.mult)
            sqv = sq.rearrange("p (s e) -> p s e", e=11)
            stv = stats_sb.rearrange("p (s e) -> p s e", e=11)
            usum = sc.tile([128, NSLOT], f32)
            nc.vector.tensor_reduce(usum[:], sqv[:, :, 0:10], AX.X, A.add)
            var = sc.tile([128, NSLOT], f32)
            nc.vector.tensor_tensor(var[:], usum[:], sqv[:, :, 10], A.subtract)
            eps = sc.tile([128, 1], f32)
            nc.vector.memset(eps[:], 1e-5)
            sd = sc.tile([128, NSLOT], f32)
            nc.scalar.activation(sd[:], var[:], AF.Sqrt, bias=eps[:])
            rstd = sc.tile([128, NSLOT], f32)
            nc.vector.reciprocal(rstd[:], sd[:])
            # scaled features f18..f27 = geom * rstd ; f28 = -mu*rstd
            for i in range(10):
                eng_i = nc.vector if i % 2 == 0 else nc.gpsimd
                eng_i.tensor_tensor(TFb[:, :, 18 + i], TFb[:, :, i],
                                    rstd[:, 0:JB], A.mult)
            nc.vector.scalar_tensor_tensor(TFb[:, :, 28], stv[:, 0:JB, 10],
                                           -1.0, rstd[:, 0:JB], A.mult, A.mult)

            # -------- remaining dist tokens ---------------------------------
            for dk in range(24, JD):
                dist_iter(dk)

        with tc.tile_pool(name="ctps2", bufs=2, space="PSUM") as ctp:
            # -------- T2: re-transpose box chunks; weave z_n block 0 in ----
            for ci in range(NCH2):
                ps = ctp.tile([128, 128], bf16, tag="ct")
                nc.tensor.transpose(ps[:], TF[:, ci * 128:(ci + 1) * 128], idb)
                dst = cta2[:, ci * 128:(ci + 1) * 128]
                if ci % 3 == 2:
                    nc.scalar.copy(dst, ps[:])
                else:
                    nc.vector.tensor_copy(dst, ps[:])
                if 4 <= ci < 12:
                    zn_mm(0, ci - 4)

        # -------- main box loop; z_n of block b+1 woven into block b --------
        # Three slots' accumulation groups are interleaved so consecutive PE
        # instructions hit different PSUM banks and pipeline-overlap.
        with tc.tile_pool(name="ops", bufs=5, space="PSUM") as op:
            box_stage = None
            gstart = 0
            eng = 0
            for j0 in range(0, JB, 3):
                grp = (j0, j0 + 1, j0 + 2)
                ot = {j: op.tile([128, D], f32, tag="o", name=f"o{j}")
                      for j in grp}
                for j in grp:
                    nc.tensor.matmul(ot[j][:], hT[0][:, j * 128:(j + 1) * 128],
                                     w2hi, start=True, stop=False)
                for j in grp:
                    nc.tensor.matmul(ot[j][:], hT[1][:, j * 128:(j + 1) * 128],
                                     w2lo, start=False, stop=False)
                for j in grp:
                    ci, jj = j // 4, j % 4
                    cam = (j % 6) // 2
                    nc.tensor.matmul(ot[j][:], cta[32 * jj:32 * jj + 32,
                                               ci * 128:(ci + 1) * 128],
                                     w2x[cam][32 * jj:32 * jj + 32, :],
                                     start=False, stop=True,
                                     tile_position=(32 * jj, 0))
                for j in grp:
                    if j % 2 == 0 and j // 16 + 1 < NBLK:
                        zn_mm(j // 16 + 1, (j % 16) // 2)
                for j in grp:
                    if box_stage is None:
                        box_stage = bstg.tile([128, 8 * D], bf16, tag="bstage")
                        gstart = j
                    off = j - gstart
                    dst = box_stage[:, off * D:(off + 1) * D]
                    if eng == 0:
                        nc.vector.tensor_copy(dst, ot[j][:])
                    else:
                        nc.scalar.copy(dst, ot[j][:])
                    eng = (eng + 1) % 2
                    if off == 7 or j == JB - 1 or (j >= 144 and off >= 2):
                        nc.sync.dma_start(vb[:, :, gstart: j + 1, :],
                                          box_stage[:, : (off + 1) * D])
                        box_stage = None

    nc.compile()
    return nc


def _prep_inputs(inputs):
    f32 = np.float32
    bf = ml_dtypes.bfloat16
    scale = float(np.asarray(inputs["scale"]))

    # W1 with the cx/cy doubling folded in (features store 2*cx, 2*cy)
    W1p = np.asarray(inputs["geom_w1"], f32).copy()
    W1p[6] *= 0.5
    W1p[7] *= 0.5

    # W1n: rows f18..f27 -> W1p, f28 -> ones (the -mu*rstd passthrough)
    W1n = np.zeros((32, DH), f32)
    W1n[18:28] = W1p
    W1n[28] = 1.0
    w1n_cols = np.concatenate([W1n[:, :128], W1n[:, 128:]], axis=1)  # [32, 256]
    w1n_rep = np.tile(w1n_cols, (4, 1))

    W2s = scale * np.asarray(inputs["geom_w2"], f32)
    w2hi, w2lo = W2s[:128], W2s[128:]

    cat_t = np.asarray(inputs["cat_table"], f32)
    cam_t = np.asarray(inputs["cam_table"], f32)
    bias_row = (np.asarray(inputs["geom_b2"], f32)
                + np.asarray(inputs["conf_b"], f32)
                + np.asarray(inputs["center_b"], f32))
    w2x_reps = []
    for c in range(3):
        W2x = np.zeros((32, D), f32)
        W2x[6] = scale * np.asarray(inputs["center_w"], f32)[0] * 0.5
        W2x[7] = scale * np.asarray(inputs["center_w"], f32)[1] * 0.5
        W2x[10:13] = scale * cat_t
        W2x[13] = scale * np.asarray(inputs["conf_w"], f32)[0]
        W2x[14] = scale * (bias_row + cam_t[c])
        W2x[15] = np.asarray(inputs["missing_emb"], f32)[0]
        W2x[16] = np.asarray(inputs["dist_w"], f32)[0]
        W2x[17] = np.asarray(inputs["dist_b"], f32)
        w2x_reps.append(np.tile(W2x, (4, 1)))

    # stats: L = chol(W1p W1p^T / 256), m = W1p.sum(1)/256;
    # block-diag rhs [128, 44]: rows 32j'+f (f<10) -> cols 11j'+(L[f,:]|m[f])
    G = (W1p @ W1p.T) / float(DH)
    Lc = np.linalg.cholesky(G + 1e-12 * np.eye(10))
    m = W1p.sum(axis=1) / float(DH)
    LM = np.zeros((128, 44), f32)
    for jp in range(4):
        LM[32 * jp:32 * jp + 10, 11 * jp:11 * jp + 10] = Lc
        LM[32 * jp:32 * jp + 10, 11 * jp + 10] = m

    idf32 = np.eye(128, dtype=f32)
    bpk = np.concatenate(
        [idf32, w1n_rep, w2hi, w2lo] + w2x_reps + [LM], axis=1
    ).astype(bf)

    box = np.asarray(inputs["box_data"], f32)
    fpks = []
    for c in range(NCORES):
        rawc = box[c * BPC:(c + 1) * BPC].reshape(BPC, T * 6, 6)
        rawc = rawc.reshape(BPC, 8, JB, 6).reshape(128, 900)
        fpks.append(np.ascontiguousarray(rawc, dtype=f32))
    return fpks, bpk


def _fast_path_ok(inputs):
    try:
        shapes = {
            "box_data": (B, T, 6, 6), "cat_table": (3, D), "geom_w1": (10, DH),
            "geom_b1": (DH,), "ln_g": (DH,), "ln_b": (DH,), "geom_w2": (DH, D),
            "geom_b2": (D,), "conf_w": (1, D), "conf_b": (D,),
            "center_w": (2, D), "center_b": (D,), "missing_emb": (1, D),
            "dist_w": (1, D), "dist_b": (D,), "cam_table": (NCAM, D),
        }
        for k, s in shapes.items():
            if tuple(np.asarray(inputs[k]).shape) != s:
                return False
        if not np.all(np.asarray(inputs["geom_b1"]) == 0):
            return False
        if not np.all(np.asarray(inputs["ln_g"]) == 1):
            return False
        if not np.all(np.asarray(inputs["ln_b"]) == 0):
            return False
        return True
    except Exception:
        return False


def _numpy_fallback(inputs):
    # Exact (slow) port of the reference for unexpected inputs.
    import math
    f32 = np.float32
    inp = {k: np.asarray(v) for k, v in inputs.items()}
    coords = inp["box_data"][..., :4].astype(f32)
    category = inp["box_data"][..., 4].astype(np.int32)
    conf = inp["box_data"][..., 5].astype(f32)
    norm = np.array([IW, IH, IW, IH], f32)
    cn = (coords / norm).reshape(B, T, NCAM, NB, 4)
    category = category.reshape(B, T, NCAM, NB)
    conf = conf.reshape(B, T, NCAM, NB, 1)
    presence = (cn.sum(-1) != 0).astype(f32)
    sort_key = category.astype(f32) + (1.0 - presence) * 1000.0
    idx = np.argsort(sort_key, axis=-1, kind="stable")
    cn = np.take_along_axis(cn, idx[..., None], axis=-2)
    category = np.take_along_axis(category, idx, axis=-1)
    conf = np.take_along_axis(conf, idx[..., None], axis=-2)
    presence = (cn.sum(-1) != 0).astype(f32)[..., None]
    x1, y1, x2, y2 = cn[..., 0], cn[..., 1], cn[..., 2], cn[..., 3]
    w, h = x2 - x1, y2 - y1
    cx, cy = (x1 + x2) * 0.5, (y1 + y2) * 0.5
    area, aspect = w * h, w / (h + 1e-6)
    dx, dy = cx[..., 0] - cx[..., 1], cy[..., 0] - cy[..., 1]
    dist = np.sqrt(dx * dx + dy * dy)[..., None]
    dist_tok = dist @ inp["dist_w"].astype(f32) + inp["dist_b"].astype(f32)
    geom = np.stack([x1, y1, x2, y2, w, h, cx, cy, area, aspect], axis=-1)
    z = geom @ inp["geom_w1"].astype(f32) + inp["geom_b1"].astype(f32)
    mu = z.mean(-1, keepdims=True)
    var = ((z - mu) ** 2).mean(-1, keepdims=True)
    xh = (z - mu) / np.sqrt(var + 1e-5) * inp["ln_g"].astype(f32) + inp["ln_b"].astype(f32)
    try:
        from scipy.special import erf as _erf
        g = xh * 0.5 * (1.0 + _erf(xh / np.sqrt(2.0)))
    except Exception:
        verf = np.vectorize(math.erf)
        g = xh * 0.5 * (1.0 + verf(xh / np.sqrt(2.0)))
    geom_p = g @ inp["geom_w2"].astype(f32) + inp["geom_b2"].astype(f32)
    cat_emb = inp["cat_table"].astype(f32)[category]
    conf_p = conf @ inp["conf_w"].astype(f32) + inp["conf_b"].astype(f32)
    center_p = np.stack([cx, cy], axis=-1) @ inp["center_w"].astype(f32) + inp["center_b"].astype(f32)
    cam_emb = inp["cam_table"].astype(f32).reshape(1, 1, NCAM, 1, D)
    tok = (geom_p + cat_emb + conf_p + center_p + cam_emb) * float(inp["scale"])
    tok = np.where(presence == 0, inp["missing_emb"].astype(f32)[0], tok)
    out = np.concatenate([dist_tok.reshape(B, T * NCAM, D),
                          tok.reshape(B, T * NCAM * NB, D)], axis=1)
    return out.astype(np.float32)


def _run(inputs, trace=False, tmpdir=None):
    from concourse.bass_utils import run_bass_kernel_spmd

    if "nc" not in _CACHE:
        _CACHE["nc"] = _build_nc()
    nc = _CACHE["nc"]

    fpks, bpk = _prep_inputs(inputs)
    in_maps = [{"fpk": fpks[c], "bpk": bpk} for c in range(NCORES)]
    res = run_bass_kernel_spmd(nc, in_maps, list(range(NCORES)),
                               trace=trace, tmpdir=tmpdir)
    out = np.concatenate(
        [np.concatenate([np.asarray(res.results[c]["outd"]),
                         np.asarray(res.results[c]["outb"])], axis=1)
         for c in range(NCORES)], axis=0)
    return out.astype(np.float32, copy=False), res


def kernel(**inputs):
    if not _fast_path_ok(inputs):
        return _numpy_fallback(inputs)
    out, _ = _run(inputs)
    return out


if __name__ == "__main__":
    import reference as ref
    inputs = {k: np.asarray(v) for k, v in ref.setup_inputs().items()}
    got = kernel(**inputs)
    exp = np.load("/tmp/expected.npy")
    d = got - exp
    print("rel fro:", np.linalg.norm(d) / np.linalg.norm(exp))
    print("absmax rel:", np.abs(d).max() / np.abs(exp).max())
